# revision 6
# baseline (speedup 1.0000x reference)
"""Batched Viterbi decode (CRF.forward) on 8 Trainium2 NeuronCores.

Problem: feats [1024, 512, 64] f32, transitions [64, 64] f32 ->
         (path_score [1024] f32, best_path [1024, 512] int32)

Sharding: data-parallel over batch B: 128 sequences per core (= the 128 SBUF
partitions), transitions replicated. The scan over T stays local per core.

Per-core kernel (all fp32, bit-exact vs the jax reference):
  forward, per t:  scores[b,(n,p)] = trans_tiled[(n,p)] + fv[b,p]   (DVE add,
                   0-stride broadcast AP), M = segmented max (tensor_reduce
                   axis=X over 3D AP), fv_t = M + feat_t (written into an
                   SBUF-resident fv history [128, T*64]).
  terminal:        term = fv_{T-1} + trans[STOP,:]; path_score = max(term);
                   best_tag = min{j: term_j == max}  (first-index argmax,
                   matching jnp.argmax tie-breaking).
  backtrack, per t: onehot(cur) --PE transpose--> onehotT --PE matmul with
                   trans--> R[b,:] = trans[cur_b,:]; cand = R + fv_{t-1};
                   path[t-1] = min{j: cand_j == max(cand)}.
"""

import numpy as np

K = 64
T = 512
B = 1024
NCORES = 8
START_IX = 62
STOP_IX = 63
NEG = -10000.0
BIG = 65536.0

_CACHE = {}


def _build_nc():
    import concourse.bass as bass
    import concourse.tile as tile
    from concourse import bacc, mybir

    F32 = mybir.dt.float32
    I32 = mybir.dt.int32

    nc = bacc.Bacc("TRN2", target_bir_lowering=False, debug=False)

    feats = nc.dram_tensor("feats", [T, 128, K], F32, kind="ExternalInput")
    trans_tiled_d = nc.dram_tensor("trans_tiled", [128, K * K], F32, kind="ExternalInput")
    trans_nat_d = nc.dram_tensor("trans_nat", [K, K], F32, kind="ExternalInput")
    iota_d = nc.dram_tensor("iota64", [128, K], F32, kind="ExternalInput")
    iotapb_d = nc.dram_tensor("iotapb", [128, K], F32, kind="ExternalInput")
    init_d = nc.dram_tensor("init_fv", [128, K], F32, kind="ExternalInput")
    ident_d = nc.dram_tensor("ident", [128, 128], F32, kind="ExternalInput")

    psc_out = nc.dram_tensor("path_score", [128, 1], F32, kind="ExternalOutput")
    path_out = nc.dram_tensor("best_path", [128, T], I32, kind="ExternalOutput")

    with tile.TileContext(nc) as tc:
        with (
            tc.tile_pool(name="const", bufs=1) as cpool,
            tc.tile_pool(name="hist", bufs=1) as hpool,
            tc.tile_pool(name="scores", bufs=1) as spool,
            tc.tile_pool(name="feat", bufs=3) as fpool,
            tc.tile_pool(name="small", bufs=2) as mpool,
            tc.tile_pool(name="path", bufs=1) as ppool,
            tc.tile_pool(name="psum", bufs=2, space="PSUM") as pspool,
        ):
            trans_tiled = cpool.tile([128, K * K], F32, tag="trans_tiled")
            nc.sync.dma_start(trans_tiled[:], trans_tiled_d.ap())
            trans_nat = cpool.tile([K, K], F32, tag="trans_nat")
            nc.sync.dma_start(trans_nat[:], trans_nat_d.ap())
            iota64 = cpool.tile([128, K], F32, tag="iota64")
            nc.sync.dma_start(iota64[:], iota_d.ap())
            iotarb = cpool.tile([128, K], F32, tag="iotarb")
            nc.sync.dma_start(iotarb[:], iotapb_d.ap())
            init_fv = cpool.tile([128, K], F32, tag="init_fv")
            nc.sync.dma_start(init_fv[:], init_d.ap())
            ident = cpool.tile([128, 128], F32, tag="ident")
            nc.sync.dma_start(ident[:], ident_d.ap())

            hist = hpool.tile([128, T * K], F32, tag="hist")
            scores = spool.tile([128, K * K], F32, tag="scores")
            scores3d = scores[:].rearrange("p (n k) -> p n k", k=K)
            tt3d = trans_tiled[:].rearrange("p (n k) -> p n k", k=K)
            msc = mpool.tile([128, K], F32, tag="msc")

            # ---------------- forward ----------------
            for t in range(T):
                ft = fpool.tile([128, K], F32, tag="feat")
                nc.sync.dma_start(ft[:], feats.ap()[t])

                prev = init_fv[:] if t == 0 else hist[:, (t - 1) * K : t * K]
                prev_b = bass.AP(prev.tensor, prev.offset, [prev.ap[0], [0, K], prev.ap[1]])
                nc.vector.tensor_add(scores3d, tt3d, prev_b)
                nc.vector.tensor_reduce(
                    msc[:], scores3d, axis=mybir.AxisListType.X, op=mybir.AluOpType.max
                )
                nc.vector.tensor_add(hist[:, t * K : (t + 1) * K], msc[:], ft[:])

            # ---------------- terminal ----------------
            term = mpool.tile([128, K], F32, tag="term")
            nc.vector.tensor_add(
                term[:], hist[:, (T - 1) * K : T * K], trans_tiled[:, STOP_IX * K : (STOP_IX + 1) * K]
            )
            psc = mpool.tile([128, 1], F32, tag="psc")
            nc.vector.tensor_reduce(
                psc[:], term[:], axis=mybir.AxisListType.X, op=mybir.AluOpType.max
            )
            nc.sync.dma_start(psc_out.ap(), psc[:])

            pathbuf = ppool.tile([128, T], F32, tag="pathbuf")
            idxm = mpool.tile([128, K], F32, tag="idxm")

            def argmin_match(cand_ap, v_ap, out_ap):
                # out = BIG - min{j : cand[j] == v} : exact first-index argmax
                # in reversed encoding (iotarb[j] = BIG - j); decoded at the end.
                nc.vector.scalar_tensor_tensor(
                    out=idxm[:],
                    in0=cand_ap,
                    scalar=v_ap,
                    in1=iotarb[:],
                    op0=mybir.AluOpType.is_equal,
                    op1=mybir.AluOpType.mult,
                )
                nc.vector.tensor_reduce(
                    out_ap, idxm[:], axis=mybir.AxisListType.X, op=mybir.AluOpType.max
                )

            argmin_match(term[:], psc[:], pathbuf[:, T - 1 : T])

            onehot = mpool.tile([128, K], F32, tag="onehot")
            cand = mpool.tile([128, K], F32, tag="cand")
            vmax = mpool.tile([128, 1], F32, tag="vmax")

            # ---------------- backtrack ----------------
            for t in range(T - 1, 0, -1):
                nc.vector.tensor_scalar(
                    out=onehot[:],
                    in0=iotarb[:],
                    scalar1=pathbuf[:, t : t + 1],
                    scalar2=None,
                    op0=mybir.AluOpType.is_equal,
                )
                ohT_ps = pspool.tile([K, 128], F32, tag="ohT_ps")
                nc.tensor.transpose(ohT_ps[:], onehot[:], ident[:])
                ohT = mpool.tile([K, 128], F32, tag="ohT")
                nc.scalar.copy(ohT[:], ohT_ps[:])
                r_ps = pspool.tile([128, K], F32, tag="r_ps")
                nc.tensor.matmul(r_ps[:], ohT[:], trans_nat[:], start=True, stop=True)
                nc.vector.tensor_add(cand[:], r_ps[:], hist[:, (t - 1) * K : t * K])
                nc.vector.tensor_reduce(
                    vmax[:], cand[:], axis=mybir.AxisListType.X, op=mybir.AluOpType.max
                )
                argmin_match(cand[:], vmax[:], pathbuf[:, t - 1 : t])

            path_i = ppool.tile([128, T], I32, tag="path_i")
            # decode reversed encoding: tag = BIG - stored
            nc.vector.tensor_scalar(
                out=path_i[:],
                in0=pathbuf[:],
                scalar1=-1.0,
                scalar2=BIG,
                op0=mybir.AluOpType.mult,
                op1=mybir.AluOpType.add,
            )
            nc.sync.dma_start(path_out.ap(), path_i[:])

    nc.compile()
    return nc


def _get_nc():
    if "nc" not in _CACHE:
        _CACHE["nc"] = _build_nc()
    return _CACHE["nc"]


def _make_core_inputs(feats_core, transitions):
    """feats_core: [128, T, K] f32 (one core's batch shard)."""
    trans_flat = transitions.reshape(-1).astype(np.float32)
    iota = np.broadcast_to(np.arange(K, dtype=np.float32), (128, K)).copy()
    init = np.full((128, K), NEG, np.float32)
    init[:, START_IX] = 0.0
    return {
        "feats": np.ascontiguousarray(feats_core.transpose(1, 0, 2)),
        "trans_tiled": np.broadcast_to(trans_flat, (128, K * K)).copy(),
        "trans_nat": np.ascontiguousarray(transitions.astype(np.float32)),
        "iota64": iota,
        "iotapb": BIG - iota,
        "init_fv": init,
        "ident": np.eye(128, dtype=np.float32),
    }


def kernel(feats, transitions):
    feats = np.asarray(feats, dtype=np.float32)
    transitions = np.asarray(transitions, dtype=np.float32)
    assert feats.shape == (B, T, K), feats.shape
    assert transitions.shape == (K, K), transitions.shape

    from concourse.bass_utils import run_bass_kernel_spmd

    nc = _get_nc()
    bc = B // NCORES
    in_maps = [
        _make_core_inputs(feats[c * bc : (c + 1) * bc], transitions)
        for c in range(NCORES)
    ]
    res = run_bass_kernel_spmd(nc, in_maps, core_ids=list(range(NCORES)))
    path_score = np.concatenate(
        [res.results[c]["path_score"].reshape(-1) for c in range(NCORES)]
    ).astype(np.float32)
    best_path = np.concatenate(
        [res.results[c]["best_path"] for c in range(NCORES)], axis=0
    ).astype(np.int32)
    return path_score, best_path


# revision 7
# speedup vs baseline: 1.0255x; 1.0255x over previous
"""Batched Viterbi decode (CRF.forward) on 8 Trainium2 NeuronCores.

Problem: feats [1024, 512, 64] f32, transitions [64, 64] f32 ->
         (path_score [1024] f32, best_path [1024, 512] int32)

Sharding: data-parallel over batch B: 128 sequences per core (= the 128 SBUF
partitions), transitions replicated. The scan over T stays local per core.

Per-core kernel (all fp32, bit-exact vs the jax reference):
  forward, per t:  scores[b,(n,p)] = trans_tiled[(n,p)] + fv[b,p]   (DVE add,
                   0-stride broadcast AP), M = segmented max (tensor_reduce
                   axis=X over 3D AP), fv_t = M + feat_t (written into an
                   SBUF-resident fv history [128, T*64]).
  terminal:        term = fv_{T-1} + trans[STOP,:]; path_score = max(term);
                   best_tag = min{j: term_j == max}  (first-index argmax,
                   matching jnp.argmax tie-breaking).
  backtrack, per t: onehot(cur) --PE transpose--> onehotT --PE matmul with
                   trans--> R[b,:] = trans[cur_b,:]; cand = R + fv_{t-1};
                   path[t-1] = min{j: cand_j == max(cand)}.
"""

import numpy as np

K = 64
T = 512
B = 1024
NCORES = 8
START_IX = 62
STOP_IX = 63
NEG = -10000.0
BIG = 65536.0

_CACHE = {}


def _build_nc():
    import concourse.bass as bass
    import concourse.tile as tile
    from concourse import bacc, mybir

    F32 = mybir.dt.float32
    I32 = mybir.dt.int32

    nc = bacc.Bacc("TRN2", target_bir_lowering=False, debug=False)

    feats = nc.dram_tensor("feats", [T, 128, K], F32, kind="ExternalInput")
    trans_tiled_d = nc.dram_tensor("trans_tiled", [128, K * K], F32, kind="ExternalInput")
    trans_nat_d = nc.dram_tensor("trans_nat", [K, K], F32, kind="ExternalInput")
    iota_d = nc.dram_tensor("iota64", [128, K], F32, kind="ExternalInput")
    iotapb_d = nc.dram_tensor("iotapb", [128, K], F32, kind="ExternalInput")
    init_d = nc.dram_tensor("init_fv", [128, K], F32, kind="ExternalInput")
    ident_d = nc.dram_tensor("ident", [128, 128], F32, kind="ExternalInput")

    psc_out = nc.dram_tensor("path_score", [128, 1], F32, kind="ExternalOutput")
    path_out = nc.dram_tensor("best_path", [128, T], I32, kind="ExternalOutput")

    with tile.TileContext(nc) as tc:
        with (
            tc.tile_pool(name="const", bufs=1) as cpool,
            tc.tile_pool(name="hist", bufs=1) as hpool,
            tc.tile_pool(name="scores", bufs=1) as spool,
            tc.tile_pool(name="feat", bufs=3) as fpool,
            tc.tile_pool(name="small", bufs=2) as mpool,
            tc.tile_pool(name="path", bufs=1) as ppool,
            tc.tile_pool(name="psum", bufs=2, space="PSUM") as pspool,
        ):
            trans_tiled = cpool.tile([128, K * K], F32, tag="trans_tiled")
            nc.sync.dma_start(trans_tiled[:], trans_tiled_d.ap())
            trans_nat = cpool.tile([K, K], F32, tag="trans_nat")
            nc.sync.dma_start(trans_nat[:], trans_nat_d.ap())
            iota64 = cpool.tile([128, K], F32, tag="iota64")
            nc.sync.dma_start(iota64[:], iota_d.ap())
            iotarb = cpool.tile([128, K], F32, tag="iotarb")
            nc.sync.dma_start(iotarb[:], iotapb_d.ap())
            init_fv = cpool.tile([128, K], F32, tag="init_fv")
            nc.sync.dma_start(init_fv[:], init_d.ap())
            ident = cpool.tile([128, 128], F32, tag="ident")
            nc.sync.dma_start(ident[:], ident_d.ap())

            hist = hpool.tile([128, T * K], F32, tag="hist")
            scores = spool.tile([128, K * K], F32, tag="scores")
            scores3d = scores[:].rearrange("p (n k) -> p n k", k=K)
            tt3d = trans_tiled[:].rearrange("p (n k) -> p n k", k=K)
            msc = mpool.tile([128, K], F32, tag="msc")

            # ---------------- forward ----------------
            # t=0: init makes START (62) the unique argmax for every next-tag,
            # so fv_0[n] = trans[n, START] + feat_0[n] exactly (no reduce).
            # For t>=1, prev tags START (fv ~ -1e4) and STOP (column masked to
            # -1e4) trail every other candidate by ~1e4 and can never win any
            # argmax, so the score/reduce width shrinks from 64 to 62.
            KP = 62
            ft0 = fpool.tile([128, K], F32, tag="feat")
            nc.sync.dma_start(ft0[:], feats.ap()[0])
            tta = trans_tiled[:]
            tcol_start = bass.AP(tta.tensor, tta.offset + START_IX, [tta.ap[0], [K, K]])
            nc.vector.tensor_add(hist[:, 0:K], tcol_start, ft0[:])

            sc62 = bass.AP(
                scores[:].tensor, scores[:].offset, [scores[:].ap[0], [KP, K], [1, KP]]
            )
            tt62 = bass.AP(tta.tensor, tta.offset, [tta.ap[0], [K, K], [1, KP]])
            for t in range(1, T):
                ft = fpool.tile([128, K], F32, tag="feat")
                nc.sync.dma_start(ft[:], feats.ap()[t])

                prev = hist[:, (t - 1) * K : t * K]
                prev_b = bass.AP(prev.tensor, prev.offset, [prev.ap[0], [0, K], [1, KP]])
                nc.vector.tensor_add(sc62, tt62, prev_b)
                nc.vector.tensor_reduce(
                    msc[:], sc62, axis=mybir.AxisListType.X, op=mybir.AluOpType.max
                )
                nc.vector.tensor_add(hist[:, t * K : (t + 1) * K], msc[:], ft[:])

            # ---------------- terminal ----------------
            term = mpool.tile([128, K], F32, tag="term")
            nc.vector.tensor_add(
                term[:], hist[:, (T - 1) * K : T * K], trans_tiled[:, STOP_IX * K : (STOP_IX + 1) * K]
            )
            psc = mpool.tile([128, 1], F32, tag="psc")
            nc.vector.tensor_reduce(
                psc[:], term[:], axis=mybir.AxisListType.X, op=mybir.AluOpType.max
            )
            nc.sync.dma_start(psc_out.ap(), psc[:])

            pathbuf = ppool.tile([128, T], F32, tag="pathbuf")
            idxm = mpool.tile([128, K], F32, tag="idxm")

            def argmin_match(cand_ap, v_ap, out_ap):
                # out = BIG - min{j : cand[j] == v} : exact first-index argmax
                # in reversed encoding (iotarb[j] = BIG - j); decoded at the end.
                nc.vector.scalar_tensor_tensor(
                    out=idxm[:],
                    in0=cand_ap,
                    scalar=v_ap,
                    in1=iotarb[:],
                    op0=mybir.AluOpType.is_equal,
                    op1=mybir.AluOpType.mult,
                )
                nc.vector.tensor_reduce(
                    out_ap, idxm[:], axis=mybir.AxisListType.X, op=mybir.AluOpType.max
                )

            argmin_match(term[:], psc[:], pathbuf[:, T - 1 : T])

            onehot = mpool.tile([128, K], F32, tag="onehot")
            cand = mpool.tile([128, K], F32, tag="cand")
            vmax = mpool.tile([128, 1], F32, tag="vmax")

            # ---------------- backtrack ----------------
            for t in range(T - 1, 0, -1):
                nc.vector.tensor_scalar(
                    out=onehot[:],
                    in0=iotarb[:],
                    scalar1=pathbuf[:, t : t + 1],
                    scalar2=None,
                    op0=mybir.AluOpType.is_equal,
                )
                ohT_ps = pspool.tile([K, 128], F32, tag="ohT_ps")
                nc.tensor.transpose(ohT_ps[:], onehot[:], ident[:])
                ohT = mpool.tile([K, 128], F32, tag="ohT")
                nc.scalar.copy(ohT[:], ohT_ps[:])
                r_ps = pspool.tile([128, K], F32, tag="r_ps")
                nc.tensor.matmul(r_ps[:], ohT[:], trans_nat[:], start=True, stop=True)
                nc.vector.tensor_add(cand[:], r_ps[:], hist[:, (t - 1) * K : t * K])
                nc.vector.tensor_reduce(
                    vmax[:], cand[:], axis=mybir.AxisListType.X, op=mybir.AluOpType.max
                )
                argmin_match(cand[:], vmax[:], pathbuf[:, t - 1 : t])

            path_i = ppool.tile([128, T], I32, tag="path_i")
            # decode reversed encoding: tag = BIG - stored
            nc.vector.tensor_scalar(
                out=path_i[:],
                in0=pathbuf[:],
                scalar1=-1.0,
                scalar2=BIG,
                op0=mybir.AluOpType.mult,
                op1=mybir.AluOpType.add,
            )
            nc.sync.dma_start(path_out.ap(), path_i[:])

    nc.compile()
    return nc


def _get_nc():
    if "nc" not in _CACHE:
        _CACHE["nc"] = _build_nc()
    return _CACHE["nc"]


def _make_core_inputs(feats_core, transitions):
    """feats_core: [128, T, K] f32 (one core's batch shard)."""
    trans_flat = transitions.reshape(-1).astype(np.float32)
    iota = np.broadcast_to(np.arange(K, dtype=np.float32), (128, K)).copy()
    init = np.full((128, K), NEG, np.float32)
    init[:, START_IX] = 0.0
    return {
        "feats": np.ascontiguousarray(feats_core.transpose(1, 0, 2)),
        "trans_tiled": np.broadcast_to(trans_flat, (128, K * K)).copy(),
        "trans_nat": np.ascontiguousarray(transitions.astype(np.float32)),
        "iota64": iota,
        "iotapb": BIG - iota,
        "init_fv": init,
        "ident": np.eye(128, dtype=np.float32),
    }


def kernel(feats, transitions):
    feats = np.asarray(feats, dtype=np.float32)
    transitions = np.asarray(transitions, dtype=np.float32)
    assert feats.shape == (B, T, K), feats.shape
    assert transitions.shape == (K, K), transitions.shape

    from concourse.bass_utils import run_bass_kernel_spmd

    nc = _get_nc()
    bc = B // NCORES
    in_maps = [
        _make_core_inputs(feats[c * bc : (c + 1) * bc], transitions)
        for c in range(NCORES)
    ]
    res = run_bass_kernel_spmd(nc, in_maps, core_ids=list(range(NCORES)))
    path_score = np.concatenate(
        [res.results[c]["path_score"].reshape(-1) for c in range(NCORES)]
    ).astype(np.float32)
    best_path = np.concatenate(
        [res.results[c]["best_path"] for c in range(NCORES)], axis=0
    ).astype(np.int32)
    return path_score, best_path


# revision 10
# speedup vs baseline: 1.0339x; 1.0083x over previous
"""Batched Viterbi decode (CRF.forward) on 8 Trainium2 NeuronCores.

Problem: feats [1024, 512, 64] f32, transitions [64, 64] f32 ->
         (path_score [1024] f32, best_path [1024, 512] int32)

Sharding: data-parallel over batch B: 128 sequences per core (= the 128 SBUF
partitions), transitions replicated. The scan over T stays local per core.

Per-core kernel (all fp32, bit-exact vs the jax reference):
  forward, per t:  scores[b,(n,p)] = trans_tiled[(n,p)] + fv[b,p]   (DVE add,
                   0-stride broadcast AP), M = segmented max (tensor_reduce
                   axis=X over 3D AP), fv_t = M + feat_t (written into an
                   SBUF-resident fv history [128, T*64]).
  terminal:        term = fv_{T-1} + trans[STOP,:]; path_score = max(term);
                   best_tag = min{j: term_j == max}  (first-index argmax,
                   matching jnp.argmax tie-breaking).
  backtrack, per t: onehot(cur) --PE transpose--> onehotT --PE matmul with
                   trans--> R[b,:] = trans[cur_b,:]; cand = R + fv_{t-1};
                   path[t-1] = min{j: cand_j == max(cand)}.
"""

import numpy as np

K = 64
T = 512
B = 1024
NCORES = 8
START_IX = 62
STOP_IX = 63
NEG = -10000.0
BIG = 65536.0

_CACHE = {}


def _build_nc():
    import concourse.bass as bass
    import concourse.tile as tile
    from concourse import bacc, mybir

    F32 = mybir.dt.float32
    I32 = mybir.dt.int32

    nc = bacc.Bacc("TRN2", target_bir_lowering=False, debug=False)

    feats = nc.dram_tensor("feats", [T, 128, K], F32, kind="ExternalInput")
    trans_tiled_d = nc.dram_tensor("trans_tiled", [128, K * K], F32, kind="ExternalInput")
    trans_nat_d = nc.dram_tensor("trans_nat", [K, K], F32, kind="ExternalInput")
    iota_d = nc.dram_tensor("iota64", [128, K], F32, kind="ExternalInput")
    iotapb_d = nc.dram_tensor("iotapb", [128, K], F32, kind="ExternalInput")
    init_d = nc.dram_tensor("init_fv", [128, K], F32, kind="ExternalInput")
    ident_d = nc.dram_tensor("ident", [128, 128], F32, kind="ExternalInput")

    psc_out = nc.dram_tensor("path_score", [128, 1], F32, kind="ExternalOutput")
    path_out = nc.dram_tensor("best_path", [128, T], I32, kind="ExternalOutput")
    # pre-feat maxes M_t, staged to HBM during forward; the backtrack gathers
    # M_t[b, c_b] as the exact argmax comparand, off the critical path.
    m_hist = nc.dram_tensor("m_hist", [T, 128, K], F32, kind="Internal")

    with tile.TileContext(nc) as tc:
        with (
            tc.tile_pool(name="const", bufs=1) as cpool,
            tc.tile_pool(name="hist", bufs=1) as hpool,
            tc.tile_pool(name="scores", bufs=1) as spool,
            tc.tile_pool(name="feat", bufs=3) as fpool,
            tc.tile_pool(name="small", bufs=2) as mpool,
            tc.tile_pool(name="path", bufs=1) as ppool,
            tc.tile_pool(name="psum", bufs=2, space="PSUM") as pspool,
        ):
            trans_tiled = cpool.tile([128, K * K], F32, tag="trans_tiled")
            nc.sync.dma_start(trans_tiled[:], trans_tiled_d.ap())
            trans_nat = cpool.tile([K, K], F32, tag="trans_nat")
            nc.sync.dma_start(trans_nat[:], trans_nat_d.ap())
            iota64 = cpool.tile([128, K], F32, tag="iota64")
            nc.sync.dma_start(iota64[:], iota_d.ap())
            iotarb = cpool.tile([128, K], F32, tag="iotarb")
            nc.sync.dma_start(iotarb[:], iotapb_d.ap())
            init_fv = cpool.tile([128, K], F32, tag="init_fv")
            nc.sync.dma_start(init_fv[:], init_d.ap())
            ident = cpool.tile([128, 128], F32, tag="ident")
            nc.sync.dma_start(ident[:], ident_d.ap())

            hist = hpool.tile([128, T * K], F32, tag="hist")
            scores = spool.tile([128, K * K], F32, tag="scores")
            scores3d = scores[:].rearrange("p (n k) -> p n k", k=K)
            tt3d = trans_tiled[:].rearrange("p (n k) -> p n k", k=K)
            msc = mpool.tile([128, K], F32, tag="msc")

            # ---------------- forward ----------------
            # t=0: init makes START (62) the unique argmax for every next-tag,
            # so fv_0[n] = trans[n, START] + feat_0[n] exactly (no reduce).
            # For t>=1, prev tags START (fv ~ -1e4) and STOP (column masked to
            # -1e4) trail every other candidate by ~1e4 and can never win any
            # argmax, so the score/reduce width shrinks from 64 to 62.
            KP = 62
            ft0 = fpool.tile([128, K], F32, tag="feat")
            nc.sync.dma_start(ft0[:], feats.ap()[0])
            tta = trans_tiled[:]
            tcol_start = bass.AP(tta.tensor, tta.offset + START_IX, [tta.ap[0], [K, K]])
            nc.vector.tensor_add(hist[:, 0:K], tcol_start, ft0[:])

            sc62 = bass.AP(
                scores[:].tensor, scores[:].offset, [scores[:].ap[0], [KP, K], [1, KP]]
            )
            tt62 = bass.AP(tta.tensor, tta.offset, [tta.ap[0], [K, K], [1, KP]])
            for t in range(1, T):
                ft = fpool.tile([128, K], F32, tag="feat")
                nc.sync.dma_start(ft[:], feats.ap()[t])

                prev = hist[:, (t - 1) * K : t * K]
                prev_b = bass.AP(prev.tensor, prev.offset, [prev.ap[0], [0, K], [1, KP]])
                nc.vector.tensor_add(sc62, tt62, prev_b)
                mt = fpool.tile([128, K], F32, tag="msc_r")
                nc.vector.tensor_reduce(
                    mt[:], sc62, axis=mybir.AxisListType.X, op=mybir.AluOpType.max
                )
                nc.sync.dma_start(m_hist.ap()[t], mt[:])
                nc.vector.tensor_add(hist[:, t * K : (t + 1) * K], mt[:], ft[:])

            # ---------------- terminal ----------------
            term = mpool.tile([128, K], F32, tag="term")
            nc.vector.tensor_add(
                term[:], hist[:, (T - 1) * K : T * K], trans_tiled[:, STOP_IX * K : (STOP_IX + 1) * K]
            )
            psc = mpool.tile([128, 1], F32, tag="psc")
            nc.vector.tensor_reduce(
                psc[:], term[:], axis=mybir.AxisListType.X, op=mybir.AluOpType.max
            )
            nc.sync.dma_start(psc_out.ap(), psc[:])

            pathbuf = ppool.tile([128, T], F32, tag="pathbuf")
            idxm = mpool.tile([128, K], F32, tag="idxm")

            def argmin_match(cand_ap, v_ap, out_ap):
                # out = BIG - min{j : cand[j] == v} : exact first-index argmax
                # in reversed encoding (iotarb[j] = BIG - j); decoded at the end.
                nc.vector.scalar_tensor_tensor(
                    out=idxm[:],
                    in0=cand_ap,
                    scalar=v_ap,
                    in1=iotarb[:],
                    op0=mybir.AluOpType.is_equal,
                    op1=mybir.AluOpType.mult,
                )
                nc.vector.tensor_reduce(
                    out_ap, idxm[:], axis=mybir.AxisListType.X, op=mybir.AluOpType.max
                )

            argmin_match(term[:], psc[:], pathbuf[:, T - 1 : T])

            onehot = mpool.tile([128, K], F32, tag="onehot")
            cand = mpool.tile([128, K], F32, tag="cand")
            vmax = mpool.tile([128, 1], F32, tag="vmax")

            # ---------------- backtrack ----------------
            vgm = mpool.tile([128, K], F32, tag="vgm")
            for t in range(T - 1, 0, -1):
                # off-critical-path: gather V = M_t[b, c_b] (the exact max of
                # this step's candidates) from the prefetched M history.
                mrt = fpool.tile([128, K], F32, tag="mring")
                nc.sync.dma_start(mrt[:], m_hist.ap()[t])
                nc.vector.scalar_tensor_tensor(
                    out=vgm[:],
                    in0=iotarb[:],
                    scalar=pathbuf[:, t : t + 1],
                    in1=mrt[:],
                    op0=mybir.AluOpType.is_equal,
                    op1=mybir.AluOpType.mult,
                )
                nc.vector.tensor_reduce(
                    vmax[:], vgm[:], axis=mybir.AxisListType.X, op=mybir.AluOpType.add
                )
                # critical path: one-hot -> PE transpose -> ACT copy -> PE gather
                nc.vector.tensor_scalar(
                    out=onehot[:],
                    in0=iotarb[:],
                    scalar1=pathbuf[:, t : t + 1],
                    scalar2=None,
                    op0=mybir.AluOpType.is_equal,
                )
                ohT_ps = pspool.tile([K, 128], F32, tag="ohT_ps")
                nc.tensor.transpose(ohT_ps[:], onehot[:], ident[:])
                ohT = mpool.tile([K, 128], F32, tag="ohT")
                nc.scalar.copy(ohT[:], ohT_ps[:])
                r_ps = pspool.tile([128, K], F32, tag="r_ps")
                nc.tensor.matmul(r_ps[:], ohT[:], trans_nat[:], start=True, stop=True)
                nc.vector.tensor_add(cand[:], r_ps[:], hist[:, (t - 1) * K : t * K])
                argmin_match(cand[:], vmax[:], pathbuf[:, t - 1 : t])

            path_i = ppool.tile([128, T], I32, tag="path_i")
            # decode reversed encoding: tag = BIG - stored
            nc.vector.tensor_scalar(
                out=path_i[:],
                in0=pathbuf[:],
                scalar1=-1.0,
                scalar2=BIG,
                op0=mybir.AluOpType.mult,
                op1=mybir.AluOpType.add,
            )
            nc.sync.dma_start(path_out.ap(), path_i[:])

    nc.compile()
    return nc


def _get_nc():
    if "nc" not in _CACHE:
        _CACHE["nc"] = _build_nc()
    return _CACHE["nc"]


def _make_core_inputs(feats_core, transitions):
    """feats_core: [128, T, K] f32 (one core's batch shard)."""
    trans_flat = transitions.reshape(-1).astype(np.float32)
    iota = np.broadcast_to(np.arange(K, dtype=np.float32), (128, K)).copy()
    init = np.full((128, K), NEG, np.float32)
    init[:, START_IX] = 0.0
    return {
        "feats": np.ascontiguousarray(feats_core.transpose(1, 0, 2)),
        "trans_tiled": np.broadcast_to(trans_flat, (128, K * K)).copy(),
        "trans_nat": np.ascontiguousarray(transitions.astype(np.float32)),
        "iota64": iota,
        "iotapb": BIG - iota,
        "init_fv": init,
        "ident": np.eye(128, dtype=np.float32),
    }


def kernel(feats, transitions):
    feats = np.asarray(feats, dtype=np.float32)
    transitions = np.asarray(transitions, dtype=np.float32)
    assert feats.shape == (B, T, K), feats.shape
    assert transitions.shape == (K, K), transitions.shape

    from concourse.bass_utils import run_bass_kernel_spmd

    nc = _get_nc()
    bc = B // NCORES
    in_maps = [
        _make_core_inputs(feats[c * bc : (c + 1) * bc], transitions)
        for c in range(NCORES)
    ]
    res = run_bass_kernel_spmd(nc, in_maps, core_ids=list(range(NCORES)))
    path_score = np.concatenate(
        [res.results[c]["path_score"].reshape(-1) for c in range(NCORES)]
    ).astype(np.float32)
    best_path = np.concatenate(
        [res.results[c]["best_path"] for c in range(NCORES)], axis=0
    ).astype(np.int32)
    return path_score, best_path
